# revision 30
# baseline (speedup 1.0000x reference)
"""Multi-head graph-attention layer for Trainium2 (8-core SPMD).

The reference computes per-head projections hp = einsum("bnf,hfd->bhnd", h, W),
dense attention scores e = hp @ hp^T, LeakyReLU, softmax over the last axis,
and then multiplies hp by sum_j(softmax(e))_j. The sum of a softmax over its
own normalization axis is identically 1, so the layer's exact mathematical
output is hp itself (concatenated over heads):

    out[b, n, h*64+d] = sum_f h[b,n,f] * W[h,f,d]  =  (h[b] @ Wc)[n, h*64+d]

with Wc[f, h*64+d] = W[h,f,d]. The reference's deviation from rowsum==1 is
fp32 rounding noise (~1e-6 relative) that no reimplementation reproduces, so
computing the projection directly is both the fastest and the most accurate
realization. `adj` is unused by the reference and is ignored here.

Sharding: data-parallel over the batch dim B=8, one graph per NeuronCore.
Each core computes Y[b]^T = (Wc^T @ h[b]^T) as a [256,256] x [256,2048]
matmul with Wc chunks stationary on the PE (float32r: single-pass reduced-
precision fp32 at 1 cycle/row, measured rel err 1.4e-4 vs 2.2e-3 for bf16).

Pipeline (per core, HW-trace-tuned):
- Host packs [Wc | X^T] row-wise so each k-chunk streams as single-run
  contiguous DMAs on the sync HWDGE queue (~400 GB/s once ramped); a tiny
  dummy read pulls the queue out of its ~100 GB/s slow-start earlier.
- k-chunk input halves are interleaved so the first node-half's
  accumulation groups close while the second half streams in; their
  output DMAs overlap the input tail.
- Scratch warm-up matmuls run during the DMA wait and short filler
  matmuls plug PE idle gaps, holding the HAM clock at 2.4 GHz (cold
  matmuls are 2x slower).
- PSUM eviction alternates DVE/ACT; each engine's chunks fly out on its
  own HWDGE queue (sync issues DVE's chunks) to avoid cross-engine
  stalls and split the ~370 GB/s write bandwidth.
"""

import numpy as np

import concourse.bass as bass
import concourse.mybir as mybir
import concourse.tile as tile
from concourse import bacc
from concourse.bass_utils import run_bass_kernel_spmd

B = 8          # graphs == cores
N = 2048       # nodes per graph
F_IN = 256     # input features (= contraction dim K)
F_OUT = 256    # num_heads * d_head
P = 128        # SBUF/PSUM partitions
NTILE = 512    # PSUM bank free-dim (fp32)

KC = F_IN // P     # 2 contraction chunks
MC = F_OUT // P    # 2 output-feature chunks
NC_ = N // NTILE   # 4 node chunks
XSPLIT = 2         # node-dim halves per x DMA
XW = N // XSPLIT   # 1024

N_WARMUP_MM = 8    # scratch matmuls covering the input-DMA wait

# PE matmul dtype: float32 (exact, 4 cycles/row), float32r (reduced-precision
# single pass, 1 cycle/row at N=512, rel err ~1.4e-4), bfloat16 (1 cycle/row,
# half input DMA, rel err ~2.2e-3).
MATMUL_DTYPE = "float32r"

_module_cache = {}

# test.py reads this after calling kernel() to get profile/exec-time info.
LAST_RESULTS = None


def _build_module(mm_dtype: str) -> bass.Bass:
    if mm_dtype == "bfloat16":
        in_dt = mybir.dt.bfloat16
    elif mm_dtype == "float32r":
        in_dt = mybir.dt.float32r
    else:
        in_dt = mybir.dt.float32

    nc = bacc.Bacc(None, target_bir_lowering=False, enable_partition_id=False)
    # Host-packed input: xin[f, 0:256] = Wc[f, :], xin[f, 256:] = X^T[f, :].
    xin = nc.dram_tensor("xin", [F_IN, F_OUT + N], in_dt, kind="ExternalInput")
    yt = nc.dram_tensor("yt", [F_OUT, N], mybir.dt.float32, kind="ExternalOutput")
    XOFF = F_OUT  # x columns start here inside a packed row

    with tile.TileContext(nc) as tc:
        with (
            tc.tile_pool(name="xpool", bufs=1) as xpool,
            tc.tile_pool(name="ypool", bufs=1) as ypool,
            tc.tile_pool(name="warmpool", bufs=1) as warmpool,
            tc.tile_pool(name="pspool", bufs=1, space="PSUM") as pspool,
        ):
            # Scratch operands for PE warm-up (zeros; values are irrelevant).
            wu = warmpool.tile([P, NTILE], mybir.dt.bfloat16, name="wu", tag="wu")
            nc.gpsimd.memset(wu[:], 0.0)
            wu_mm = wu[:]

            # Per-k packed tiles [128, 2304]: cols 0:256 weights, 256: x.
            # Two single-run DMAs per k so matmuls chase the stream; all on
            # the sync HWDGE queue in need-order.
            CUT = F_OUT + XW  # per-k split: [w | x first half], then rest
            xk_sb = [
                xpool.tile([P, F_OUT + N], in_dt, name=f"xk{k}", tag=f"xk{k}")
                for k in range(KC)
            ]
            # Tiny dummy read first: pulls the HWDGE queue + DRAM path out
            # of its slow-start earlier so the real stream ramps sooner.
            dummy = xpool.tile([P, 32], in_dt, name="dummy", tag="dummy")
            nc.sync.dma_start(dummy[:], xin[0:P, :32])
            # Split the input across both HWDGE queues: the last compute
            # segment's chunks (n3) ride the otherwise-idle scalar queue in
            # parallel with the main stream, so when the PE reaches that
            # segment its data has long landed - no input wait on the tail.
            # Main (sync) queue, k-interleaved, in need-order: w+n0n1, n2.
            SPLITS = [0, XOFF + 2 * NTILE, XOFF + 3 * NTILE]
            for lo, hi in zip(SPLITS[:-1], SPLITS[1:]):
                nc.sync.dma_start(xk_sb[0][:, lo:hi], xin[0:P, lo:hi])
                nc.sync.dma_start(xk_sb[1][:, lo:hi], xin[P : 2 * P, lo:hi])
            lo, hi = XOFF + 3 * NTILE, XOFF + N
            nc.scalar.dma_start(xk_sb[0][:, lo:hi], xin[0:P, lo:hi])
            nc.scalar.dma_start(xk_sb[1][:, lo:hi], xin[P : 2 * P, lo:hi])


            ps = [
                [
                    pspool.tile(
                        [P, NTILE], mybir.dt.float32, name=f"ps{m}_{n}", tag=f"ps{m}_{n}"
                    )
                    for n in range(NC_)
                ]
                for m in range(MC)
            ]
            y_sb = [
                ypool.tile([P, N], mybir.dt.float32, name=f"y{m}", tag=f"y{m}")
                for m in range(MC)
            ]

            # PE clock warm-up on scratch data while the x DMAs are in
            # flight. Runs on ps[0][0] before its real accumulation group;
            # Tile's WAW tracking keeps program order.
            for _ in range(N_WARMUP_MM):
                nc.tensor.matmul(ps[0][0][:], wu_mm[:, :P], wu_mm, start=True, stop=True)

            def filler(target, count):
                # Keep the PE activity window busy during input-wait gaps so
                # the HAM clock stays at 2.4 GHz. Targets a PSUM bank that is
                # either already evicted or about to be reset by start=True -
                # never one mid-accumulation.
                for _ in range(count):
                    nc.tensor.matmul(target[:], wu_mm[:, :P], wu_mm, start=True, stop=True)

            # Segment outer, then k: each segment's groups close right
            # after its k=1 chunk lands, so outputs fly while later
            # segments are still streaming in.
            SEGS = [[0, 1], [2], [3]]
            for si, seg in enumerate(SEGS):
                for k in range(KC):
                  # ps[1][3] is untouched until the last segment resets it;
                  # ps[0][0] is the first bank evicted, done long before.
                  filler(ps[0][0] if si == len(SEGS) - 1 else ps[1][3], 2)
                  for n in seg:
                    for m in range(MC):
                        nc.tensor.matmul(
                            ps[m][n][:],
                            xk_sb[k][:, m * P : (m + 1) * P],
                            xk_sb[k][:, XOFF + n * NTILE : XOFF + (n + 1) * NTILE],
                            start=(k == 0),
                            stop=(k == KC - 1),
                        )
                        if k == KC - 1:
                            # Eviction alternates DVE/ACT; all outputs go on
                            # the single warmed-up HWDGE stream.
                            dst = y_sb[m][:, n * NTILE : (n + 1) * NTILE]
                            yslice = yt[m * P : (m + 1) * P, n * NTILE : (n + 1) * NTILE]
                            if (2 * n + m) % 2 == 0:
                                # DVE evicts; the idle sync engine issues the
                                # store on its (ramped) queue.
                                nc.vector.tensor_copy(dst, ps[m][n][:])
                                nc.sync.dma_start(yslice, dst)
                            else:
                                # ACT evicts and issues its own store on the
                                # scalar queue - no cross-engine stall.
                                nc.scalar.copy(dst, ps[m][n][:])
                                nc.scalar.dma_start(yslice, dst)
    nc.compile()
    return nc


def _get_module() -> bass.Bass:
    if MATMUL_DTYPE not in _module_cache:
        _module_cache[MATMUL_DTYPE] = _build_module(MATMUL_DTYPE)
    return _module_cache[MATMUL_DTYPE]


def kernel(h: np.ndarray, adj: np.ndarray, W: np.ndarray, **_unused) -> np.ndarray:
    global LAST_RESULTS
    h = np.asarray(h, dtype=np.float32)
    W = np.asarray(W, dtype=np.float32)
    # Wc[f, head*64+d] = W[head, f, d]
    wc = np.ascontiguousarray(W.transpose(1, 0, 2).reshape(F_IN, F_OUT))

    if MATMUL_DTYPE == "bfloat16":
        import ml_dtypes

        cast = lambda a: np.ascontiguousarray(a.astype(ml_dtypes.bfloat16))
    else:
        cast = np.ascontiguousarray

    wc_in = cast(wc)
    in_maps = [
        {"xin": np.ascontiguousarray(np.hstack([wc_in, cast(h[b].T)]))}
        for b in range(B)
    ]
    nc = _get_module()
    res = run_bass_kernel_spmd(nc, in_maps, core_ids=list(range(B)))
    LAST_RESULTS = res

    out = np.empty((B, N, F_OUT), dtype=np.float32)
    for b in range(B):
        out[b] = res.results[b]["yt"].T
    return out


# revision 31
# speedup vs baseline: 1.0667x; 1.0667x over previous
"""Multi-head graph-attention layer for Trainium2 (8-core SPMD).

The reference computes per-head projections hp = einsum("bnf,hfd->bhnd", h, W),
dense attention scores e = hp @ hp^T, LeakyReLU, softmax over the last axis,
and then multiplies hp by sum_j(softmax(e))_j. The sum of a softmax over its
own normalization axis is identically 1, so the layer's exact mathematical
output is hp itself (concatenated over heads):

    out[b, n, h*64+d] = sum_f h[b,n,f] * W[h,f,d]  =  (h[b] @ Wc)[n, h*64+d]

with Wc[f, h*64+d] = W[h,f,d]. The reference's deviation from rowsum==1 is
fp32 rounding noise (~1e-6 relative) that no reimplementation reproduces, so
computing the projection directly is both the fastest and the most accurate
realization. `adj` is unused by the reference and is ignored here.

Sharding: data-parallel over the batch dim B=8, one graph per NeuronCore.
Each core computes Y[b]^T = (Wc^T @ h[b]^T) as a [256,256] x [256,2048]
matmul with Wc chunks stationary on the PE (float32r: single-pass reduced-
precision fp32 at 1 cycle/row, measured rel err 1.4e-4 vs 2.2e-3 for bf16).

Pipeline (per core, HW-trace-tuned):
- Host packs [Wc | X^T] row-wise so each k-chunk streams as single-run
  contiguous DMAs on the sync HWDGE queue (~400 GB/s once ramped); a tiny
  dummy read pulls the queue out of its ~100 GB/s slow-start earlier.
- k-chunk input halves are interleaved so the first node-half's
  accumulation groups close while the second half streams in; their
  output DMAs overlap the input tail.
- Scratch warm-up matmuls run during the DMA wait and short filler
  matmuls plug PE idle gaps, holding the HAM clock at 2.4 GHz (cold
  matmuls are 2x slower).
- PSUM eviction alternates DVE/ACT; each engine's chunks fly out on its
  own HWDGE queue (sync issues DVE's chunks) to avoid cross-engine
  stalls and split the ~370 GB/s write bandwidth.
"""

import numpy as np

import concourse.bass as bass
import concourse.mybir as mybir
import concourse.tile as tile
from concourse import bacc
from concourse.bass_utils import run_bass_kernel_spmd

B = 8          # graphs == cores
N = 2048       # nodes per graph
F_IN = 256     # input features (= contraction dim K)
F_OUT = 256    # num_heads * d_head
P = 128        # SBUF/PSUM partitions
NTILE = 512    # PSUM bank free-dim (fp32)

KC = F_IN // P     # 2 contraction chunks
MC = F_OUT // P    # 2 output-feature chunks
NC_ = N // NTILE   # 4 node chunks
XSPLIT = 2         # node-dim halves per x DMA
XW = N // XSPLIT   # 1024

N_WARMUP_MM = 8    # scratch matmuls covering the input-DMA wait

# PE matmul dtype: float32 (exact, 4 cycles/row), float32r (reduced-precision
# single pass, 1 cycle/row at N=512, rel err ~1.4e-4), bfloat16 (1 cycle/row,
# half input DMA, rel err ~2.2e-3).
MATMUL_DTYPE = "float32r"

_module_cache = {}

# test.py reads this after calling kernel() to get profile/exec-time info.
LAST_RESULTS = None


def _build_module(mm_dtype: str) -> bass.Bass:
    if mm_dtype == "bfloat16":
        in_dt = mybir.dt.bfloat16
    elif mm_dtype == "float32r":
        in_dt = mybir.dt.float32r
    else:
        in_dt = mybir.dt.float32

    nc = bacc.Bacc(None, target_bir_lowering=False, enable_partition_id=False)
    # Host-packed input: xin[f, 0:256] = Wc[f, :], xin[f, 256:] = X^T[f, :].
    xin = nc.dram_tensor("xin", [F_IN, F_OUT + N], in_dt, kind="ExternalInput")
    yt = nc.dram_tensor("yt", [F_OUT, N], mybir.dt.float32, kind="ExternalOutput")
    XOFF = F_OUT  # x columns start here inside a packed row

    with tile.TileContext(nc) as tc:
        with (
            tc.tile_pool(name="xpool", bufs=1) as xpool,
            tc.tile_pool(name="ypool", bufs=1) as ypool,
            tc.tile_pool(name="warmpool", bufs=1) as warmpool,
            tc.tile_pool(name="pspool", bufs=1, space="PSUM") as pspool,
        ):
            # Scratch operands for PE warm-up (zeros; values are irrelevant).
            wu = warmpool.tile([P, NTILE], mybir.dt.bfloat16, name="wu", tag="wu")
            nc.gpsimd.memset(wu[:], 0.0)
            wu_mm = wu[:]

            # Per-k packed tiles [128, 2304]: cols 0:256 weights, 256: x.
            # Two single-run DMAs per k so matmuls chase the stream; all on
            # the sync HWDGE queue in need-order.
            CUT = F_OUT + XW  # per-k split: [w | x first half], then rest
            xk_sb = [
                xpool.tile([P, F_OUT + N], in_dt, name=f"xk{k}", tag=f"xk{k}")
                for k in range(KC)
            ]
            # Tiny dummy read first: pulls the HWDGE queue + DRAM path out
            # of its slow-start earlier so the real stream ramps sooner.
            dummy = xpool.tile([P, 32], in_dt, name="dummy", tag="dummy")
            nc.sync.dma_start(dummy[:], xin[0:P, :32])
            # k-interleaved input stream on one queue (splitting across
            # queues only divides the shared ~390 GB/s and slows the
            # critical chunks). Chunk sizes shrink toward the end: big
            # chunks amortize DMA overhead early, tiny last chunks keep the
            # final MM->copy->out dependency chain short.
            SPLITS = [0, XOFF + 2 * NTILE, XOFF + 3 * NTILE,
                      XOFF + 3 * NTILE + NTILE // 2, XOFF + N]
            for lo, hi in zip(SPLITS[:-2], SPLITS[1:-1]):
                nc.sync.dma_start(xk_sb[0][:, lo:hi], xin[0:P, lo:hi])
                nc.sync.dma_start(xk_sb[1][:, lo:hi], xin[P : 2 * P, lo:hi])
            lo, hi = SPLITS[-2], SPLITS[-1]
            nc.sync.dma_start(xk_sb[0][:, lo:hi], xin[0:P, lo:hi])
            nc.sync.dma_start(xk_sb[1][:, lo:hi], xin[P : 2 * P, lo:hi])


            ps = [
                [
                    pspool.tile(
                        [P, NTILE], mybir.dt.float32, name=f"ps{m}_{n}", tag=f"ps{m}_{n}"
                    )
                    for n in range(NC_)
                ]
                for m in range(MC)
            ]
            y_sb = [
                ypool.tile([P, N], mybir.dt.float32, name=f"y{m}", tag=f"y{m}")
                for m in range(MC)
            ]

            # PE clock warm-up on scratch data while the x DMAs are in
            # flight. Runs on ps[0][0] before its real accumulation group;
            # Tile's WAW tracking keeps program order.
            for _ in range(N_WARMUP_MM):
                nc.tensor.matmul(ps[0][0][:], wu_mm[:, :P], wu_mm, start=True, stop=True)

            def filler(target, count):
                # Keep the PE activity window busy during input-wait gaps so
                # the HAM clock stays at 2.4 GHz. Targets a PSUM bank that is
                # either already evicted or about to be reset by start=True -
                # never one mid-accumulation.
                for _ in range(count):
                    nc.tensor.matmul(target[:], wu_mm[:, :P], wu_mm, start=True, stop=True)

            # Segment outer, then k: each segment's groups close right
            # after its k=1 chunk lands, so outputs fly while later
            # segments are still streaming in.
            SEGS = [[0, 1], [2], [3]]
            for si, seg in enumerate(SEGS):
                for k in range(KC):
                  # ps[1][3] is untouched until the last segment resets it;
                  # ps[0][0] is the first bank evicted, done long before.
                  filler(ps[0][0] if si == len(SEGS) - 1 else ps[1][3], 2)
                  for n in seg:
                    for m in range(MC):
                        nc.tensor.matmul(
                            ps[m][n][:],
                            xk_sb[k][:, m * P : (m + 1) * P],
                            xk_sb[k][:, XOFF + n * NTILE : XOFF + (n + 1) * NTILE],
                            start=(k == 0),
                            stop=(k == KC - 1),
                        )
                        if k == KC - 1:
                            # Eviction alternates DVE/ACT; all outputs go on
                            # the single warmed-up HWDGE stream.
                            dst = y_sb[m][:, n * NTILE : (n + 1) * NTILE]
                            yslice = yt[m * P : (m + 1) * P, n * NTILE : (n + 1) * NTILE]
                            if (2 * n + m) % 2 == 0:
                                # DVE evicts; the idle sync engine issues the
                                # store on its (ramped) queue.
                                nc.vector.tensor_copy(dst, ps[m][n][:])
                                nc.sync.dma_start(yslice, dst)
                            else:
                                # ACT evicts and issues its own store on the
                                # scalar queue - no cross-engine stall.
                                nc.scalar.copy(dst, ps[m][n][:])
                                nc.scalar.dma_start(yslice, dst)
    nc.compile()
    return nc


def _get_module() -> bass.Bass:
    if MATMUL_DTYPE not in _module_cache:
        _module_cache[MATMUL_DTYPE] = _build_module(MATMUL_DTYPE)
    return _module_cache[MATMUL_DTYPE]


def kernel(h: np.ndarray, adj: np.ndarray, W: np.ndarray, **_unused) -> np.ndarray:
    global LAST_RESULTS
    h = np.asarray(h, dtype=np.float32)
    W = np.asarray(W, dtype=np.float32)
    # Wc[f, head*64+d] = W[head, f, d]
    wc = np.ascontiguousarray(W.transpose(1, 0, 2).reshape(F_IN, F_OUT))

    if MATMUL_DTYPE == "bfloat16":
        import ml_dtypes

        cast = lambda a: np.ascontiguousarray(a.astype(ml_dtypes.bfloat16))
    else:
        cast = np.ascontiguousarray

    wc_in = cast(wc)
    in_maps = [
        {"xin": np.ascontiguousarray(np.hstack([wc_in, cast(h[b].T)]))}
        for b in range(B)
    ]
    nc = _get_module()
    res = run_bass_kernel_spmd(nc, in_maps, core_ids=list(range(B)))
    LAST_RESULTS = res

    out = np.empty((B, N, F_OUT), dtype=np.float32)
    for b in range(B):
        out[b] = res.results[b]["yt"].T
    return out
